# revision 26
# baseline (speedup 1.0000x reference)
"""Chamfer-style point loss (nn_PointLoss) on 8 Trainium2 NeuronCores.

Math (reference): reflect points across plane n.x+d=0; half1 = reflected
points (valid where s=p.n+d < 0, mask m1), half2 = original points (mask
m2 = ~m1). D[i,j] = ||half1[i]-half2[j]||^2. Output scalar =
50*(sum_j min_i(D) m2_j / c2 + sum_i min_j(D) m1_i / c1).

v4 formulation: the reflection is affine, R = M.p + t with the
Householder matrix M = I - 2 n n^T / |n|^2 and t = -2d n/|n|^2, so
  F[i,j] = p_i . (-2 M p_j) + rowterm_i + colterm_j
with rowterm_i = |p_i|^2 + (4d/nn) s_i + PEN*(1-m1_i)
     colterm_j = |p_j|^2 + (4d/nn) s_j - 4d^2/nn + PEN*m1_j.
A-image K-rows 0..2 are the raw transposed points (host relayout);
row 3 = rowterm (one PE transpose + DRAM flatten); row 4 = ones.
B-image rows 0..2 = -2M.pcol (one 3x3 x 3x512 matmul); row 3 = ones;
row 4 = colterm. One K=5 fp32r matmul per (128,512) tile; mins taken
directly in min-space (fp16), col-min partition reduction via PE
transposes + batched X-reduce. Cross-core combine: AllReduce(min) on a
(128,33) fp16 payload with the s1 slot trick.

Sharding: half2 (column) axis split 8 ways, 512 cols/core; every core
holds all rows. Row index i=32p+q maps to A-column j=q*128+p (tile q,
partition p); col point 512c+4p+s maps to B-column 128s+p.
"""

import os
import sys

import numpy as np

for _p in ("/opt/trn_rl_repo", "/root/.axon_site/_ro/trn_rl_repo"):
    if os.path.isdir(_p) and _p not in sys.path:
        sys.path.insert(0, _p)

import concourse.bacc as bacc
import concourse.tile as tile
from concourse import mybir
from concourse.bass_utils import run_bass_kernel_spmd

FP = mybir.dt.float32
FR = mybir.dt.float32r
HF = mybir.dt.float16
AX = mybir.AxisListType
OP = mybir.AluOpType

N = 4096
NCORES = 8
QT = 32           # row tiles (tile m covers rows j in [128m, 128m+128))
QC = 4            # col slots per partition (512 columns/core)
W = QT + QC       # merged row+col working width in the [128, W] layout
PEN = float(2**14)
BIGH = 60000.0    # slot identity magnitude (max-space: -BIGH)
CMI = -60000.0    # CM init (max-space)


def _emit(tc, out_ap, norm_ap, pa_ap, aimg_ap, bimg_ap, pcol_ap, idf_ap,
          idh_ap, oh_ap, ohrow_ap, i2n_ap):
    nc = tc.nc

    psA = tc.alloc_tile_pool(name="psA", bufs=2, space="PSUM")
    psT = tc.alloc_tile_pool(name="psT", bufs=2, space="PSUM")
    per = tc.alloc_tile_pool(name="per", bufs=1)
    fsp = tc.alloc_tile_pool(name="fsp", bufs=2)
    drm = tc.alloc_tile_pool(name="drm", bufs=1, space="DRAM")

    def _t(shape, name, dt=FP):
        return per.tile(shape, dt, name=name)

    # ---- inputs to SBUF, spread across queues
    norm_sb = _t([1, 4], "norm_sb")
    nc.sync.dma_start(norm_sb[:], norm_ap[:])
    Aimg = _t([5, N], "Aimg", FR)
    nc.sync.dma_start(Aimg[:], aimg_ap[:].bitcast(FR))
    PA = _t([128, 3, W], "PA")
    nc.scalar.dma_start(PA[:], pa_ap[:])
    pcol = _t([3, 512], "pcol", FR)
    nc.scalar.dma_start(pcol[:], pcol_ap[:].bitcast(FR))
    Bimg = _t([5, 512], "Bimg", FR)
    nc.gpsimd.dma_start(Bimg[:], bimg_ap[:].bitcast(FR))
    OH = _t([128, 1], "OH")
    nc.gpsimd.dma_start(OH[:], oh_ap[:])
    ohrow = _t([1, 128], "ohrow")
    nc.gpsimd.dma_start(ohrow[:], ohrow_ap[:])
    IDF = _t([128, 128], "IDF")
    nc.sync.dma_start(IDF[:], idf_ap[:])
    IDH = _t([128, 128], "IDH", HF)
    nc.gpsimd.dma_start(IDH[:], idh_ap[:])

    # ---- early constants (off critical path)
    ones_r = _t([1, 128], "ones_r")
    nc.gpsimd.memset(ones_r[:], 1.0)
    ones_c = _t([128, 1], "ones_c")
    nc.gpsimd.memset(ones_c[:], 1.0)
    ones_ch = _t([128, 1], "ones_ch", HF)
    nc.gpsimd.memset(ones_ch[:], 1.0)
    SGNP = _t([128, W], "SGNP")
    nc.gpsimd.memset(SGNP[:, 0:QT], -PEN)
    nc.gpsimd.memset(SGNP[:, QT:W], PEN)
    ADDR = _t([128, W], "ADDR")
    nc.gpsimd.memset(ADDR[:, 0:QT], PEN)
    I2n = _t([3, 3], "I2n")
    nc.scalar.dma_start(I2n[:], i2n_ap[:])
    CMa = _t([128, 512], "CMa", HF)
    nc.gpsimd.memset(CMa[:], CMI)
    CMb = _t([128, 512], "CMb", HF)
    nc.gpsimd.memset(CMb[:], CMI)

    # ---- norm-derived row [1,8]: [n0,n1,n2,d, 4d/nn, -4d^2/nn, 4/nn, _]
    nrow = _t([1, 8], "nrow")
    nc.gpsimd.memset(nrow[:], 0.0)
    nc.scalar.copy(nrow[:, 0:4], norm_sb[:])
    nsq = _t([1, 4], "nsq")
    nc.vector.tensor_tensor(nsq[:], norm_sb[:], norm_sb[:], op=OP.mult)
    nn_ = _t([1, 1], "nn_")
    nc.vector.tensor_reduce(nn_[:], nsq[:, 0:3], axis=AX.X, op=OP.add)
    inv_nn = _t([1, 1], "inv_nn")
    nc.vector.reciprocal(inv_nn[:], nn_[:])
    nc.vector.tensor_scalar(nrow[:, 6:7], inv_nn[:], 4.0, None, op0=OP.mult)
    nc.vector.tensor_tensor(nrow[:, 4:5], nrow[:, 6:7], norm_sb[:, 3:4],
                            op=OP.mult)
    t01 = _t([1, 1], "t01")
    nc.vector.tensor_scalar(t01[:], norm_sb[:, 3:4], -1.0, None, op0=OP.mult)
    nc.vector.tensor_tensor(nrow[:, 5:6], nrow[:, 4:5], t01[:], op=OP.mult)

    # broadcast to all 128 partitions via K=1 matmul
    nb_ps = psT.tile([128, 8], FP, name="nb_ps")
    nc.tensor.matmul(nb_ps[:], ones_r[:], nrow[:], start=True, stop=True)
    NB = _t([128, 8], "NB")
    nc.scalar.copy(NB[:], nb_ps[:])

    # ---- -2M = (4/nn) n n^T - 2I ; B3 = (-2M) @ pcol
    outer_ps = psT.tile([3, 3], FP, name="outer_ps")
    nc.tensor.matmul(outer_ps[:], norm_sb[:, 0:3], norm_sb[:, 0:3],
                     start=True, stop=True)
    statf = _t([3, 3], "statf")
    nc.vector.tensor_scalar(statf[:], outer_ps[:], NB[0:3, 6:7], None,
                            op0=OP.mult)
    stat = _t([3, 3], "stat", FR)
    nc.vector.tensor_tensor(stat[:], statf[:], I2n[:], op=OP.add)
    b3_ps = psT.tile([3, 512], FP, name="b3_ps")
    nc.tensor.matmul(b3_ps[:], stat[:], pcol[:], start=True, stop=True)
    nc.scalar.copy(Bimg[0:3, :], b3_ps[:])

    # ---- per-point chain in the [128, W] layout
    s_all = _t([128, W], "s_all")
    t1_ = _t([128, W], "t1_")
    nc.scalar.mul(s_all[:], PA[:, 0, :], NB[:, 0:1])
    nc.scalar.mul(t1_[:], PA[:, 1, :], NB[:, 1:2])
    nc.vector.tensor_tensor(s_all[:], s_all[:], t1_[:], op=OP.add)
    nc.scalar.mul(t1_[:], PA[:, 2, :], NB[:, 2:3])
    nc.vector.tensor_tensor(s_all[:], s_all[:], t1_[:], op=OP.add)
    nc.vector.tensor_scalar_add(s_all[:], s_all[:], NB[:, 3:4])

    pp = _t([128, W], "pp")
    nc.vector.tensor_tensor(pp[:], PA[:, 0, :], PA[:, 0, :], op=OP.mult)
    q1 = _t([128, W], "q1")
    nc.gpsimd.tensor_tensor(q1[:], PA[:, 1, :], PA[:, 1, :], op=OP.mult)
    nc.vector.tensor_tensor(pp[:], pp[:], q1[:], op=OP.add)
    q2 = _t([128, W], "q2")
    nc.gpsimd.tensor_tensor(q2[:], PA[:, 2, :], PA[:, 2, :], op=OP.mult)
    nc.vector.tensor_tensor(pp[:], pp[:], q2[:], op=OP.add)

    M1 = _t([128, W], "M1")
    nc.vector.tensor_scalar(M1[:], s_all[:], 0.0, None, op0=OP.is_lt)
    # ADDR col region = -4d^2/nn
    ones4 = _t([128, 4], "ones4")
    nc.gpsimd.memset(ones4[:], 1.0)
    nc.scalar.mul(ADDR[:, QT:W], ones4[:], NB[:, 5:6])

    # rowterm/colterm merged: rt = pp + (4d/nn) s + M1*SGNP + ADDR
    g_ = _t([128, W], "g_")
    nc.vector.tensor_scalar(g_[:], s_all[:], NB[:, 4:5], None, op0=OP.mult)
    nc.vector.tensor_tensor(g_[:], g_[:], pp[:], op=OP.add)
    t4 = _t([128, W], "t4")
    nc.gpsimd.tensor_tensor(t4[:], M1[:], SGNP[:], op=OP.mult)
    nc.vector.tensor_tensor(g_[:], g_[:], t4[:], op=OP.add)
    rt = _t([128, W], "rt")
    nc.vector.tensor_tensor(rt[:], g_[:], ADDR[:], op=OP.add)

    # masks for the tail (fp16), off critical path
    M1h = _t([128, QT], "M1h", HF)
    nc.scalar.copy(M1h[:], M1[:, 0:QT])

    # ---- transpose rt [128,36] -> [36,128]; flatten via DRAM
    rt_ps = psT.tile([W, 128], FP, name="rt_ps")
    nc.tensor.transpose(rt_ps[:], rt[:], IDF[:])
    rt_sb = _t([W, 128], "rt_sb")
    nc.scalar.copy(rt_sb[:], rt_ps[:])
    stg = drm.tile([W, 128], FP, name="stg")
    nc.sync.dma_start(stg[:], rt_sb[:])
    nc.gpsimd.dma_start(Aimg[3:4, :], stg[0:QT, :].bitcast(FR))
    nc.sync.dma_start(Bimg[4:5, :], stg[QT:W, :].bitcast(FR))

    # ---- c1/c2 + reciprocals (overlaps the main loop)
    c1row = _t([128, 1], "c1row")
    nc.vector.tensor_reduce(c1row[:], M1[:, 0:QT], axis=AX.X, op=OP.add)
    c1_ps = psT.tile([1, 1], FP, name="c1_ps")
    nc.tensor.matmul(c1_ps[:], c1row[:], ones_c[:], start=True, stop=True)
    c1 = _t([1, 1], "c1")
    nc.scalar.copy(c1[:], c1_ps[:])
    c2 = _t([1, 1], "c2")
    nc.vector.tensor_scalar(c2[:], c1[:], -1.0, float(N), op0=OP.mult,
                            op1=OP.add)
    nc.vector.tensor_scalar_max(c1[:], c1[:], 1.0)
    nc.vector.tensor_scalar_max(c2[:], c2[:], 1.0)
    rcv = _t([1, 2], "rcv")
    nc.vector.reciprocal(rcv[:, 0:1], c1[:])
    nc.vector.reciprocal(rcv[:, 1:2], c2[:])
    bm = _t([128, 1], "bm")
    nc.vector.tensor_scalar(bm[:], OH[:], BIGH, -BIGH, op0=OP.mult,
                            op1=OP.add)

    # ---- main loop: 16 batches of 2 row tiles
    Ar = Aimg[:]
    Br = Bimg[:]
    pay = _t([128, QT + 1], "pay", HF)
    for b in range(QT // 2):
        ps = psA.tile([128, 2, 512], FP, name="ps")
        for t in range(2):
            m = 2 * b + t
            nc.tensor.matmul(ps[:, t, :], Ar[:, 128 * m : 128 * (m + 1)],
                             Br[:], start=True, stop=True)
        FS = fsp.tile([128, 2, 512], HF, name="FS")
        nc.scalar.mul(FS[:], ps[:], -1.0)
        nc.vector.tensor_reduce(pay[:, 2 * b : 2 * b + 2], FS[:], axis=AX.X,
                                op=OP.max)
        P = fsp.tile([128, 512], HF, tag="P", name="P")
        nc.vector.tensor_tensor(P[:], FS[:, 0, :], FS[:, 1, :], op=OP.max)
        CMt = CMa if b % 2 == 0 else CMb
        nc.vector.tensor_tensor(CMt[:], CMt[:], P[:], op=OP.max)

    # ---- columns: transpose CMa/CMb, batched X-reduce -> d1t [128,4]
    d1_ps = psT.tile([128, 8, 128], HF, tag="d", name="d1_ps")
    for g in range(4):
        nc.tensor.transpose(d1_ps[:, g, :], CMa[:, 128 * g : 128 * (g + 1)],
                            IDH[:])
    for g in range(4):
        nc.tensor.transpose(d1_ps[:, 4 + g, :],
                            CMb[:, 128 * g : 128 * (g + 1)], IDH[:])
    d1t8 = _t([128, 8], "d1t8", HF)
    nc.vector.tensor_reduce(d1t8[:], d1_ps[:], axis=AX.X, op=OP.max)
    d1t = _t([128, 4], "d1t", HF)
    nc.vector.tensor_tensor(d1t[:], d1t8[:, 0:4], d1t8[:, 4:8], op=OP.max)
    m2f = _t([128, QC], "m2f")
    nc.vector.tensor_scalar(m2f[:], M1[:, QT:W], -1.0, 1.0, op0=OP.mult,
                            op1=OP.add)
    m2fh = _t([128, QC], "m2fh", HF)
    nc.scalar.copy(m2fh[:], m2f[:])
    w1 = _t([128, 4], "w1", HF)
    nc.vector.tensor_tensor(w1[:], d1t[:], m2fh[:], op=OP.mult)
    w1s = _t([128, 1], "w1s")
    nc.vector.tensor_reduce(w1s[:], w1[:], axis=AX.X, op=OP.add)
    s1_ps = psT.tile([1, 1], FP, tag="t", name="s1_ps")
    nc.tensor.matmul(s1_ps[:], w1s[:], ones_c[:], start=True, stop=True)
    s1 = _t([1, 1], "s1")
    nc.scalar.copy(s1[:], s1_ps[:])

    # slot encode via one-hot ROW matmul: slot = s1*oh + (1-oh)*(-BIGH)
    s1b_ps = psT.tile([128, 1], FP, tag="t", name="s1b_ps")
    nc.tensor.matmul(s1b_ps[:], ohrow[:], s1[:], start=True, stop=True)
    slot = _t([128, 1], "slot")
    nc.vector.tensor_tensor(slot[:], s1b_ps[:], bm[:], op=OP.add)
    nc.scalar.copy(pay[:, QT : QT + 1], slot[:])

    # ---- AllReduce(max) of [D2 | slot] over all 8 cores (fp16 payload)
    pd = drm.tile([128, QT + 1], HF, name="pd")
    pd2 = nc.dram_tensor("pd2x", [128, QT + 1], HF,
                         addr_space="Shared").ap()
    nc.gpsimd.dma_start(pd[:], pay[:])
    nc.gpsimd.collective_compute(
        "AllReduce",
        OP.max,
        replica_groups=[list(range(NCORES))],
        ins=[pd.opt()],
        outs=[pd2],
    )
    G = _t([128, QT + 1], "G", HF)
    nc.gpsimd.dma_start(G[:], pd2)

    # ---- finish: s2 = sum(G2*m1h); sum slots; combine
    w2 = _t([128, QT], "w2", HF)
    nc.vector.tensor_tensor(w2[:], G[:, 0:QT], M1h[:], op=OP.mult)
    w2s = _t([128, 1], "w2s")
    nc.vector.tensor_reduce(w2s[:], w2[:], axis=AX.X, op=OP.add)
    s2_ps = psT.tile([1, 1], FP, tag="t", name="s2_ps")
    nc.tensor.matmul(s2_ps[:], w2s[:], ones_c[:], start=True, stop=True)
    sa_ps = psT.tile([1, 1], FP, tag="t", name="sa_ps")
    nc.tensor.matmul(sa_ps[:], G[0:NCORES, QT : QT + 1],
                     ones_ch[0:NCORES, :], start=True, stop=True)
    sv = _t([1, 2], "sv")
    nc.scalar.copy(sv[:, 0:1], s2_ps[:])
    nc.scalar.copy(sv[:, 1:2], sa_ps[:])
    pv = _t([1, 2], "pv")
    nc.vector.tensor_tensor(pv[:], sv[:], rcv[:], op=OP.mult)
    res = _t([1, 1], "res")
    nc.vector.tensor_reduce(res[:], pv[:], axis=AX.X, op=OP.add)
    nc.scalar.mul(res[:], res[:], -50.0)
    nc.sync.dma_start(out_ap[:], res[:])

    for p in (psA, psT, per, fsp, drm):
        p.seal()


_NC = None


def build():
    global _NC
    if _NC is not None:
        return _NC
    nc = bacc.Bacc(
        "TRN2", target_bir_lowering=False, debug=False, num_devices=NCORES
    )
    norm_ap = nc.dram_tensor("norm4", [1, 4], FP, kind="ExternalInput").ap()
    pa_ap = nc.dram_tensor("pa", [128, 3, W], FP, kind="ExternalInput").ap()
    aimg_ap = nc.dram_tensor("aimg", [5, N], FP, kind="ExternalInput").ap()
    bimg_ap = nc.dram_tensor("bimg", [5, 512], FP, kind="ExternalInput").ap()
    pcol_ap = nc.dram_tensor("pcol", [3, 512], FP, kind="ExternalInput").ap()
    idf_ap = nc.dram_tensor("idf", [128, 128], FP, kind="ExternalInput").ap()
    idh_ap = nc.dram_tensor("idh", [128, 128], HF, kind="ExternalInput").ap()
    oh_ap = nc.dram_tensor("oh", [128, 1], FP, kind="ExternalInput").ap()
    ohrow_ap = nc.dram_tensor("ohrow", [1, 128], FP, kind="ExternalInput").ap()
    i2n_ap = nc.dram_tensor("i2n", [3, 3], FP, kind="ExternalInput").ap()
    out_ap = nc.dram_tensor("out", [1, 1], FP, kind="ExternalOutput").ap()
    with tile.TileContext(nc) as tc:
        _emit(tc, out_ap, norm_ap, pa_ap, aimg_ap, bimg_ap, pcol_ap, idf_ap,
              idh_ap, oh_ap, ohrow_ap, i2n_ap)
    nc.compile()
    _NC = nc
    return nc


def make_in_maps(norm, points):
    norm = np.ascontiguousarray(norm, dtype=np.float32)
    pts = np.ascontiguousarray(points, dtype=np.float32)
    # A-image: j = q*128 + p  <->  point id 32p+q
    ptsT = pts.reshape(128, QT, 3).transpose(1, 0, 2).reshape(N, 3).T
    aimg = np.zeros((5, N), np.float32)
    aimg[0:3] = ptsT
    aimg[4] = 1.0
    idf = np.eye(128, dtype=np.float32)
    idh = np.eye(128, dtype=np.float16)
    maps = []
    for c in range(NCORES):
        oh = np.zeros((128, 1), np.float32)
        oh[c, 0] = 1.0
        cb = pts[512 * c : 512 * (c + 1)].reshape(128, QC, 3)  # [p, s, 3]
        # pa: [p, comp, slot] with slots = 32 row slots + 4 col slots
        pa = np.concatenate(
            [pts.reshape(128, QT, 3), cb], axis=1
        ).transpose(0, 2, 1)  # [128, 3, 36]
        pa = np.ascontiguousarray(pa)
        # B columns: j = 128s + p  <->  col point 512c + 4p + s
        pcol = np.ascontiguousarray(
            cb.transpose(1, 0, 2).reshape(512, 3).T
        )  # [3, 512]
        bimg = np.zeros((5, 512), np.float32)
        bimg[3] = 1.0
        maps.append(
            {
                "norm4": norm,
                "pa": pa,
                "aimg": aimg,
                "bimg": bimg,
                "pcol": pcol,
                "idf": idf,
                "idh": idh,
                "oh": oh,
                "ohrow": oh.reshape(1, 128).copy(),
                "i2n": (-2.0 * np.eye(3)).astype(np.float32),
            }
        )
    return maps


LAST_RESULTS = None


def kernel(norm, points):
    global LAST_RESULTS
    nc = build()
    maps = make_in_maps(norm, points)
    trace = bool(os.environ.get("KERNEL_TRACE"))
    LAST_RESULTS = run_bass_kernel_spmd(
        nc, maps, list(range(NCORES)), trace=trace
    )
    out = np.asarray(LAST_RESULTS.results[0]["out"], dtype=np.float32)
    return out.reshape(())
